# revision 37
# baseline (speedup 1.0000x reference)
"""Trainium2 Bass kernel for nn_MultiHeadAttention_5308579578426.

Multi-head attention, B=2 L=4096 D=512 H=8 DK=DV=64, returning both the
projected output [B, L, DV] and the full attention matrix [H*B, L, L].

Sharding (data + head parallel, no cross-device comm):
  core d -> batch b = d // 4, heads {2*(d%4), 2*(d%4)+1}.
Each core computes its two heads' full attention rows (written straight to
HBM) and a partial fc output (summed over its heads); the host sums the
per-batch partials and adds fc_b.

Per-core algorithm (all fp32; matmuls use the full-rate float32r PE path):
  prologue: load q/k/v [L, D], transpose via PE into [d, l] layout, project
            into QT/KT [head][65, L] and V [lk, hv] with biases folded in.
  N-side  : per (head, 128-row lq tile): scores = QT.T @ KT (K=64 matmuls),
            exp via ACT with fused row-sum accumulation, normalize rows via
            DVE tensor_scalar, DMA the [128, L] attention tile to HBM.
            Also writes row 64 of QT with -ln(rowsum) (PE-transposed).
  T-side  : per (head, 512-col lq group): scoresT = KTx.T @ QTx with K=65 --
            the 65th row pair (ones, -lnS) makes the matmul emit
            scoresT - lnS, so the ACT exp directly yields normalized
            attention^T, which feeds ctx^T += V_chunk.T @ attnT_chunk.
  fc      : outT = fcw.T @ ctxT accumulated over both heads, DMA to HBM.
"""

import os
import time
from contextlib import ExitStack

import numpy as np

B, L, D, H, DK, DV = 2, 4096, 512, 8, 64, 64
P = 128
NCORES = 8
HPC = 2  # heads per core
TEMP = 8.0  # sqrt(DK)

_RUNNER_CACHE = {}
LAST_EXEC_S = None  # wall time of the device execution of the last kernel() call


def _fills(total, width):
    """Split `total` into chunks of `width` (last chunk may be smaller)."""
    out = []
    base = 0
    while base < total:
        w = min(width, total - base)
        out.append((base, w))
        base += w
    return out


def _split_multi_waits(nc):
    """The container's walrus accepts at most one sync-wait per instruction;
    Tile attaches several. Hoist all but the last wait of each instruction
    onto no-op instructions inserted immediately before it (same engine, same
    program point -- semantically identical, sequencer waits serially)."""
    from concourse import mybir

    n = 0
    for fn in nc.m.functions:
        for bb in fn.blocks:
            insts = list(bb.instructions)
            if not any(
                i.sync_info is not None and len(i.sync_info.on_wait) > 1
                for i in insts
            ):
                continue
            out = []
            for inst in insts:
                si = inst.sync_info
                if si is not None and len(si.on_wait) > 1:
                    waits = list(si.on_wait)
                    for w in waits[:-1]:
                        n += 1
                        out.append(
                            mybir.InstNoOp(
                                name=f"WSPLIT-{nc.next_id()}",
                                engine=inst.engine,
                                sync_info=mybir.SyncInfo(on_wait=[w], on_update=[]),
                                bass_nofuse=True,
                            )
                        )
                    inst.sync_info = mybir.SyncInfo(
                        on_wait=[waits[-1]], on_update=list(si.on_update)
                    )
                out.append(inst)
            bb.instructions = out
    return n


def _build_nc(L_=L, order="interleave_lag", ps_width=1536, ps_bufs=2, pro_ps=False, ld_bufs=2, attn_bufs=4, exp_bufs=3, trn_bufs=1, act_groups=0, v_inter=False, n_pre_groups=2, split_waits=True):
    import concourse.bass as bass
    import concourse.tile as tile
    from concourse import mybir
    from concourse.masks import make_identity
    from concourse.bass import ts, ds

    f32 = mybir.dt.float32
    f32r = mybir.dt.float32r
    EXP = mybir.ActivationFunctionType.Exp
    LN = mybir.ActivationFunctionType.Ln

    nT = L_ // P  # lq tiles per head
    nG = L_ // 512  # 512-wide lq groups
    nC = L_ // P  # lk chunks

    nc = bass.Bass(
        "TRN2",
        target_bir_lowering=False,
        debug=False,
        enable_asserts=False,
        num_devices=NCORES,
    )

    # q/k/v arrive pre-transposed from the host: [D, L] row-major
    q_d = nc.dram_tensor("qt", [D, L_], f32r, kind="ExternalInput").ap()
    k_d = nc.dram_tensor("kt", [D, L_], f32r, kind="ExternalInput").ap()
    v_d = nc.dram_tensor("vt", [D, L_], f32r, kind="ExternalInput").ap()
    wqt_d = nc.dram_tensor("wqt", [D, HPC * DK], f32r, kind="ExternalInput").ap()
    wkt_d = nc.dram_tensor("wkt", [D, HPC * DK], f32r, kind="ExternalInput").ap()
    wvt_d = nc.dram_tensor("wvt", [D, HPC * DV], f32r, kind="ExternalInput").ap()
    bq_d = nc.dram_tensor("bq", [HPC * DK, 1], f32, kind="ExternalInput").ap()
    bk_d = nc.dram_tensor("bk", [HPC * DK, 1], f32, kind="ExternalInput").ap()
    bv_d = nc.dram_tensor("bv", [1, HPC * DV], f32r, kind="ExternalInput").ap()
    fcwt_d = nc.dram_tensor("fcwt", [HPC * DV, DV], f32r, kind="ExternalInput").ap()

    attn_d = nc.dram_tensor("attn", [HPC, L_, L_], f32, kind="ExternalOutput").ap()
    outp_d = nc.dram_tensor("outp", [HPC, DV, L_], f32, kind="ExternalOutput").ap()

    with tile.TileContext(nc) as tc, ExitStack() as ctx:
        sync, vec, act, pe = nc.sync, nc.vector, nc.scalar, nc.tensor

        const = ctx.enter_context(tc.tile_pool(name="const", bufs=1))
        main = ctx.enter_context(tc.tile_pool(name="main", bufs=1))
        ld = ctx.enter_context(tc.tile_pool(name="ld", bufs=ld_bufs))
        trn = ctx.enter_context(tc.tile_pool(name="trn", bufs=trn_bufs))
        attnp = ctx.enter_context(tc.tile_pool(name="attnp", bufs=attn_bufs))
        expp = ctx.enter_context(tc.tile_pool(name="expp", bufs=exp_bufs))
        stat = ctx.enter_context(tc.tile_pool(name="stat", bufs=8))
        ps = ctx.enter_context(tc.tile_pool(name="ps", bufs=ps_bufs, space="PSUM"))
        misc = ctx.enter_context(tc.tile_pool(name="misc", bufs=2, space="PSUM"))

        ident = const.tile([P, P], f32, tag="ident")
        make_identity(nc, ident)
        ones1 = const.tile([1, P], f32r, tag="ones1")
        vec.memset(ones1.bitcast(f32), 1.0)

        # --- weights to SBUF ---
        wq_sb = const.tile([P, 4, HPC * DK], f32r, tag="wq")
        sync.dma_start(out=wq_sb, in_=wqt_d.rearrange("(a p) h -> p a h", p=P))
        wk_sb = const.tile([P, 4, HPC * DK], f32r, tag="wk")
        sync.dma_start(out=wk_sb, in_=wkt_d.rearrange("(a p) h -> p a h", p=P))
        wv_sb = const.tile([P, 4, HPC * DV], f32r, tag="wv")
        sync.dma_start(out=wv_sb, in_=wvt_d.rearrange("(a p) h -> p a h", p=P))
        bq_sb = const.tile([DK, HPC], f32, tag="bq")
        sync.dma_start(out=bq_sb, in_=bq_d.rearrange("(h a) x -> a (h x)", h=HPC))
        bk_sb = const.tile([DK, HPC], f32, tag="bk")
        sync.dma_start(out=bk_sb, in_=bk_d.rearrange("(h a) x -> a (h x)", h=HPC))
        bv_sb = const.tile([1, HPC * DV], f32r, tag="bv")
        sync.dma_start(out=bv_sb, in_=bv_d)
        fcw_sb = const.tile([DV, HPC, DV], f32r, tag="fcw")
        sync.dma_start(out=fcw_sb, in_=fcwt_d.rearrange("(h a) d -> a h d", h=HPC))

        # --- persistent per-head tensors ---
        # QTx[h]/KTx[h]: [64, L] Q^T / K^T (temp pre-folded into Q weights).
        # V_sb: [lk-part, chunk, head*(DV+1)] -- per head [V_h | ones]; the
        # ones column makes the ctx matmul accumulate S = rowsum(exp(scores))
        # in psum row 64.
        QTx = [main.tile([DK, L_], f32r, tag=f"qtx{h}", name=f"qtx{h}") for h in range(HPC)]
        KTx = [main.tile([DK, L_], f32r, tag=f"ktx{h}", name=f"ktx{h}") for h in range(HPC)]
        V_sb = main.tile([P, nC, HPC * (DV + 1)], f32r, tag="vsb")
        vec.memset(V_sb.bitcast(f32), 1.0)
        cstage = ctx.enter_context(tc.tile_pool(name="cstage", bufs=2))
        ostage = ctx.enter_context(tc.tile_pool(name="ostage", bufs=2))

        # [D, L] -> [p = d%128, dc = d//128, lq]
        q_r = q_d.rearrange("(a p) l -> p a l", p=P)
        k_r = k_d.rearrange("(a p) l -> p a l", p=P)
        v_r = v_d.rearrange("(a p) l -> p a l", p=P)

        def load_transpose(src_r, g, use_act=False):
            """Load the transposed-layout group [128, 4, 512]: partition = d
            within chunk dc, free = lq in the 512-wide group g."""
            gw = min(4, nT - 4 * g)
            tg = trn.tile([P, 4, 512], f32r, tag="trn")
            sync.dma_start(
                out=tg[:, :, : gw * P], in_=src_r[:, :, ds(g * 512, gw * P)]
            )
            return tg, gw

        # --- prologue: interleave K/Q groups (N-side tile t only needs the
        # K groups its fills touch + Q group t//4), then V ---
        def proj_group(src_r, g, w_sb, b_sb, dst, use_act=False):
            tg, gw = load_transpose(src_r, g, use_act=use_act)
            for h in range(HPC):
                pp = misc.tile([DK, 512], f32, tag="misc")
                for dc in range(4):
                    pe.matmul(
                        pp[:, : gw * P],
                        lhsT=w_sb[:, dc, h * DK : (h + 1) * DK],
                        rhs=tg[:, dc, : gw * P],
                        start=(dc == 0),
                        stop=(dc == 3),
                    )
                if use_act:
                    act.add(
                        out=dst[h][0:DK, ds(g * 512, gw * P)],
                        in_=pp[:, : gw * P],
                        add=b_sb[:, h : h + 1],
                    )
                else:
                    vec.tensor_scalar_add(
                        out=dst[h][0:DK, ds(g * 512, gw * P)],
                        in0=pp[:, : gw * P],
                        scalar1=b_sb[:, h : h + 1],
                    )

        def v_group(g):
            tg, gw = load_transpose(v_r, g)
            for tt in range(gw):
                c = 4 * g + tt
                pv = (ps if pro_ps else misc).tile([P, HPC * DV], f32, tag="ps" if pro_ps else "misc", name="pv")
                for dc in range(4):
                    pe.matmul(
                        pv,
                        lhsT=tg[:, dc, ts(tt, P)],
                        rhs=wv_sb[:, dc, :],
                        start=(dc == 0),
                        stop=False,
                    )
                pe.matmul(
                    pv,
                    lhsT=ones1,
                    rhs=bv_sb,
                    start=False,
                    stop=True,
                )
                for h in range(HPC):
                    vec.tensor_copy(
                        out=V_sb[:, c, h * (DV + 1) : h * (DV + 1) + DV],
                        in_=pv[:, h * DV : (h + 1) * DV],
                    )

        for g in range((nT + 3) // 4):
            proj_group(k_r, g, wk_sb, bk_sb, KTx, use_act=(g < act_groups))
            proj_group(q_r, g, wq_sb, bq_sb, QTx, use_act=(g < act_groups))
            if v_inter:
                v_group(g)
        if not v_inter:
            for g in range((nT + 3) // 4):
                v_group(g)

        n_fills = _fills(L_, ps_width)
        t_groups = _fills(nC, ps_width // 512)

        def emit_t_group(h, g):
            """T-side: scoresT = KT.T @ QT (raw), exp, ctx^T + S via the ones
            column, then normalize ctx^T, fc, and produce -lnS columns for the
            N-side tiles of this group."""
            gw4 = min(4, nT - 4 * g)
            pc = misc.tile([DV + 1, 512], f32, tag="misc", name="pc")
            for cbase, nch in t_groups:
                pt = ps.tile([P, nch * 512], f32, tag="ps", name="pt")
                et = expp.tile([P, nch * 512], f32r, tag="expp", name="et")
                for cc in range(nch):
                    c = cbase + cc
                    pe.matmul(
                        pt[:, ts(cc, 512)],
                        lhsT=KTx[h][:, ts(c, P)],
                        rhs=QTx[h][:, ts(g, 512)],
                        start=True,
                        stop=True,
                    )
                act.activation(out=et, in_=pt, func=EXP)
                for cc in range(nch):
                    c = cbase + cc
                    pe.matmul(
                        pc,
                        lhsT=V_sb[:, c, h * (DV + 1) : (h + 1) * (DV + 1)].bitcast(
                            f32r
                        ),
                        rhs=et[:, ts(cc, 512)],
                        start=(c == 0),
                        stop=(c == nC - 1),
                    )
            # stage ctx^T + S to SBUF; derive invS (transposed), -lnS, and the
            # broadcast row for normalization
            cs = cstage.tile([DV + 1, 512], f32, tag="cstage", name="cs")
            vec.tensor_copy(out=cs, in_=pc)
            pst = misc.tile([P, 4], f32, tag="misc", name="pst")
            for j in range(gw4):
                pe.transpose(
                    out=pst[:, j : j + 1],
                    in_=cs[DV : DV + 1, ts(j, P)],
                    identity=ident[DV : DV + 1, DV : DV + 1],
                )
            invS_t = stat.tile([P, 4], f32, tag="invs", bufs=4)
            vec.reciprocal(out=invS_t[:, :gw4], in_=pst[:, :gw4])
            nlnS = stat.tile([P, 4], f32, tag="nlns", bufs=4)
            act.activation(out=nlnS[:, :gw4], in_=invS_t[:, :gw4], func=LN)
            prow = misc.tile([1, 512], f32, tag="misc", name="prow")
            for j in range(gw4):
                pe.transpose(
                    out=prow[0:1, ts(j, P)], in_=invS_t[:, j : j + 1], identity=ident
                )
            invS_row = stat.tile([1, 512], f32r, tag="invsrow", bufs=2)
            vec.tensor_copy(out=invS_row[0:1, : gw4 * P], in_=prow[0:1, : gw4 * P])
            pb = misc.tile([P, 512], f32, tag="misc", name="pb")
            pe.matmul(
                pb[:, : gw4 * P],
                lhsT=ones1,
                rhs=invS_row[0:1, : gw4 * P],
                start=True,
                stop=True,
            )
            cnorm = cstage.tile([DV, 512], f32r, tag="cstage", name="cnorm")
            vec.tensor_mul(cnorm[:, : gw4 * P], cs[0:DV, : gw4 * P], pb[0:DV, : gw4 * P])
            # fc: out^T = fcw_h.T @ ctx^T
            pf = misc.tile([DV, 512], f32, tag="misc", name="pf")
            pe.matmul(
                pf[:, : gw4 * P],
                lhsT=fcw_sb[:, h, :],
                rhs=cnorm[:, : gw4 * P],
                start=True,
                stop=True,
            )
            osb = ostage.tile([DV, 512], f32, tag="ostage", name="osb")
            vec.tensor_copy(out=osb[:, : gw4 * P], in_=pf[:, : gw4 * P])
            sync.dma_start(out=outp_d[h, :, ds(g * 512, gw4 * P)], in_=osb[:, : gw4 * P])
            return nlnS

        def emit_n_tile_v1(h, t):
            """Self-sufficient N-side tile (no T-group dependency): raw exp
            with fused row-sum, then DVE normalize. Used for the first group
            so ACT has work while the V prologue finishes."""
            at = attnp.tile([P, L_], f32, tag="attn", name="at")
            sp = stat.tile([P, 8], f32, tag="stat", bufs=4)
            for f, (base, fw) in enumerate(n_fills):
                pn = ps.tile([P, fw], f32, tag="ps", name="pn")
                for jj in range(fw // 512):
                    pe.matmul(
                        pn[:, ts(jj, 512)],
                        lhsT=QTx[h][:, ts(t, P)],
                        rhs=KTx[h][:, ds(base + jj * 512, 512)],
                        start=True,
                        stop=True,
                    )
                act.activation(
                    out=at[:, ds(base, fw)],
                    in_=pn,
                    func=EXP,
                    accum_out=sp[:, f : f + 1],
                )
            nf = len(n_fills)
            if nf == 1:
                vec.reciprocal(sp[:, 5:6], sp[:, 0:1])
            else:
                vec.tensor_add(sp[:, 4:5], sp[:, 0:1], sp[:, 1:2])
                for f in range(2, nf):
                    vec.tensor_add(sp[:, 4:5], sp[:, 4:5], sp[:, f : f + 1])
                vec.reciprocal(sp[:, 5:6], sp[:, 4:5])
            vec.tensor_scalar_mul(out=at, in0=at, scalar1=sp[:, 5:6])
            sync.dma_start(out=attn_d[h, ts(t, P), :], in_=at)

        def emit_n_tile(h, t, nlnS):
            """N-side: natural scores, then exp(scores - lnS) via the ACT
            per-partition bias -> normalized attention rows, straight to HBM."""
            j = t % 4
            at = attnp.tile([P, L_], f32, tag="attn", name="at")
            for base, fw in n_fills:
                pn = ps.tile([P, fw], f32, tag="ps", name="pn")
                for jj in range(fw // 512):
                    pe.matmul(
                        pn[:, ts(jj, 512)],
                        lhsT=QTx[h][:, ts(t, P)],
                        rhs=KTx[h][:, ds(base + jj * 512, 512)],
                        start=True,
                        stop=True,
                    )
                act.activation(
                    out=at[:, ds(base, fw)],
                    in_=pn,
                    func=EXP,
                    bias=nlnS[:, j : j + 1],
                )
            sync.dma_start(out=attn_d[h, ts(t, P), :], in_=at)

        # First group's N tiles are self-sufficient (v1 style) so ACT has
        # work while V finishes; afterwards T group first (produces -lnS),
        # then its four bias-normalized N tiles.
        n_pre = min(n_pre_groups, nG)
        for h in range(HPC):
            for t in range(min(4 * n_pre, nT)):
                emit_n_tile_v1(h, t)
        for g in range(nG):
            for h in range(HPC):
                nlnS = emit_t_group(h, g)
                if g >= n_pre:
                    for t in range(4 * g, min(4 * g + 4, nT)):
                        emit_n_tile(h, t, nlnS)

    if split_waits:
        _split_multi_waits(nc)
    return nc


def _shard_inputs(inputs):
    """Full inputs -> per-core input maps (host-side, cheap)."""
    q, k, v = inputs["q"], inputs["k"], inputs["v"]
    wq, bq, wk, bk = inputs["wq"], inputs["bq"], inputs["wk"], inputs["bk"]
    wv, bv, fc_w = inputs["wv"], inputs["bv"], inputs["fc_w"]
    f32 = np.float32
    # pre-transpose per batch once (shared by the 4 cores of that batch)
    qT = [np.ascontiguousarray(q[b].T, dtype=f32) for b in range(B)]
    kT = [np.ascontiguousarray(k[b].T, dtype=f32) for b in range(B)]
    vT = [np.ascontiguousarray(v[b].T, dtype=f32) for b in range(B)]
    in_maps = []
    for d in range(NCORES):
        b = d // 4
        h0 = HPC * (d % 4)
        rows = slice(h0 * DK, (h0 + HPC) * DK)
        in_maps.append(
            {
                "qt": qT[b],
                "kt": kT[b],
                "vt": vT[b],
                "wqt": np.ascontiguousarray((wq[rows] / TEMP).T, dtype=f32),
                "wkt": np.ascontiguousarray(wk[rows].T, dtype=f32),
                "wvt": np.ascontiguousarray(wv[rows].T, dtype=f32),
                "bq": np.ascontiguousarray(
                    (bq[rows] / TEMP).reshape(HPC * DK, 1), dtype=f32
                ),
                "bk": np.ascontiguousarray(bk[rows].reshape(HPC * DK, 1), dtype=f32),
                "bv": np.ascontiguousarray(bv[rows].reshape(1, HPC * DV), dtype=f32),
                "fcwt": np.ascontiguousarray(fc_w[:, rows].T, dtype=f32),
            }
        )
    return in_maps


def _get_runner():
    """Build the Bass module once and return a cached jitted SPMD callable."""
    if "runner" in _RUNNER_CACHE:
        return _RUNNER_CACHE["runner"]

    import jax
    from jax.sharding import Mesh, PartitionSpec, NamedSharding

    try:
        from jax.experimental.shard_map import shard_map
    except ImportError:  # newer jax
        shard_map = jax.shard_map
    from concourse import bass2jax, mybir

    bass2jax.install_neuronx_cc_hook()
    nc = _build_nc()

    partition_name = nc.partition_id_tensor.name if nc.partition_id_tensor else None
    in_names, out_names, out_avals, out_shapes = [], [], [], []
    for alloc in nc.m.functions[0].allocations:
        if not isinstance(alloc, mybir.MemoryLocationSet):
            continue
        if alloc.kind not in ("ExternalInput", "ExternalOutput"):
            continue
        name = alloc.memorylocations[0].name
        if alloc.kind == "ExternalInput":
            if name != partition_name:
                in_names.append(name)
        else:
            out_names.append(name)
            shape = tuple(alloc.tensor_shape)
            dtype = mybir.dt.np(alloc.dtype)
            out_avals.append(jax.core.ShapedArray(shape, dtype))
            out_shapes.append((shape, dtype))
    n_params = len(in_names)
    all_in_names = list(in_names) + list(out_names)
    if partition_name is not None:
        all_in_names.append(partition_name)
    all_in_names = tuple(all_in_names)
    donate = tuple(range(n_params, n_params + len(out_names)))

    def _body(*args):
        operands = list(args)
        if partition_name is not None:
            operands.append(bass2jax.partition_id_tensor())
        outs = bass2jax._bass_exec_p.bind(
            *operands,
            out_avals=tuple(out_avals),
            in_names=all_in_names,
            out_names=tuple(out_names),
            lowering_input_output_aliases=(),
            sim_require_finite=True,
            sim_require_nnan=True,
            nc=nc,
        )
        return tuple(outs)

    devices = jax.devices()[:NCORES]
    assert len(devices) == NCORES, f"need {NCORES} cores, got {len(jax.devices())}"
    mesh = Mesh(np.asarray(devices), ("core",))
    in_specs = (PartitionSpec("core"),) * (n_params + len(out_names))
    out_specs = (PartitionSpec("core"),) * len(out_names)
    fn = jax.jit(
        shard_map(
            _body, mesh=mesh, in_specs=in_specs, out_specs=out_specs, check_rep=False
        ),
        donate_argnums=donate,
        keep_unused=True,
    )
    sharding = NamedSharding(mesh, PartitionSpec("core"))

    # on-device zero buffers for the donated outputs (kernel writes every
    # element, but the custom-call path wants donated operands to reuse)
    def _zeros():
        import jax.numpy as jnp

        return tuple(
            jnp.zeros((NCORES * s[0],) + tuple(s[1:]), dt) for (s, dt) in out_shapes
        )

    zeros_fn = jax.jit(_zeros, out_shardings=(sharding,) * len(out_shapes))

    runner = (jax, fn, zeros_fn, in_names, out_names, sharding)
    _RUNNER_CACHE["runner"] = runner
    return runner


def kernel(**inputs):
    global LAST_EXEC_S
    jax, fn, zeros_fn, in_names, out_names, sharding = _get_runner()

    in_maps = _shard_inputs(inputs)
    concat = [
        jax.device_put(
            np.concatenate([in_maps[c][nm] for c in range(NCORES)], axis=0), sharding
        )
        for nm in in_names
    ]
    zouts = zeros_fn()
    jax.block_until_ready((concat, zouts))

    t0 = time.perf_counter()
    outs = fn(*concat, *zouts)
    outs = jax.block_until_ready(outs)
    LAST_EXEC_S = time.perf_counter() - t0

    res = {nm: np.asarray(o) for nm, o in zip(out_names, outs)}
    # attn: [8*2, L, L]; core d rows [2d, 2d+1] are heads (2*(d%4), 2*(d%4)+1)
    # of batch d//4. attn_view[h*B + b] = global row 8*b + h.
    ga = res["attn"]
    idx = [8 * b + h for h in range(H) for b in range(B)]
    attn_view = ga[idx]
    # outp: per-core [HPC, DV, L] partial outT per head
    go = res["outp"].reshape(NCORES, HPC, DV, L)
    fc_b = np.asarray(inputs["fc_b"], dtype=np.float32)
    out = np.empty((B, L, DV), dtype=np.float32)
    for b in range(B):
        acc = go[4 * b : 4 * b + 4].sum(axis=(0, 1))  # [DV, L]
        out[b] = acc.T + fc_b
    return out, attn_view


# revision 40
# speedup vs baseline: 1.0522x; 1.0522x over previous
"""Trainium2 Bass kernel for nn_MultiHeadAttention_5308579578426.

Multi-head attention, B=2 L=4096 D=512 H=8 DK=DV=64, returning both the
projected output [B, L, DV] and the full attention matrix [H*B, L, L].

Sharding (data + head parallel, no cross-device comm):
  core d -> batch b = d // 4, heads {2*(d%4), 2*(d%4)+1}.
Each core computes its two heads' full attention rows (written straight to
HBM) and a partial fc output (summed over its heads); the host sums the
per-batch partials and adds fc_b.

Per-core algorithm (all fp32; matmuls use the full-rate float32r PE path):
  prologue: load q/k/v [L, D], transpose via PE into [d, l] layout, project
            into QT/KT [head][65, L] and V [lk, hv] with biases folded in.
  N-side  : per (head, 128-row lq tile): scores = QT.T @ KT (K=64 matmuls),
            exp via ACT with fused row-sum accumulation, normalize rows via
            DVE tensor_scalar, DMA the [128, L] attention tile to HBM.
            Also writes row 64 of QT with -ln(rowsum) (PE-transposed).
  T-side  : per (head, 512-col lq group): scoresT = KTx.T @ QTx with K=65 --
            the 65th row pair (ones, -lnS) makes the matmul emit
            scoresT - lnS, so the ACT exp directly yields normalized
            attention^T, which feeds ctx^T += V_chunk.T @ attnT_chunk.
  fc      : outT = fcw.T @ ctxT accumulated over both heads, DMA to HBM.
"""

import os
import time
from contextlib import ExitStack

import numpy as np

B, L, D, H, DK, DV = 2, 4096, 512, 8, 64, 64
P = 128
NCORES = 8
HPC = 2  # heads per core
TEMP = 8.0  # sqrt(DK)

_RUNNER_CACHE = {}
LAST_EXEC_S = None  # wall time of the device execution of the last kernel() call


def _fills(total, width):
    """Split `total` into chunks of `width` (last chunk may be smaller)."""
    out = []
    base = 0
    while base < total:
        w = min(width, total - base)
        out.append((base, w))
        base += w
    return out


def _split_multi_waits(nc):
    """The container's walrus accepts at most one sync-wait per instruction;
    Tile attaches several. Hoist all but the last wait of each instruction
    onto no-op instructions inserted immediately before it (same engine, same
    program point -- semantically identical, sequencer waits serially)."""
    from concourse import mybir

    n = 0
    for fn in nc.m.functions:
        for bb in fn.blocks:
            insts = list(bb.instructions)
            if not any(
                i.sync_info is not None and len(i.sync_info.on_wait) > 1
                for i in insts
            ):
                continue
            out = []
            for inst in insts:
                si = inst.sync_info
                if si is not None and len(si.on_wait) > 1:
                    waits = list(si.on_wait)
                    for w in waits[:-1]:
                        n += 1
                        out.append(
                            mybir.InstNoOp(
                                name=f"WSPLIT-{nc.next_id()}",
                                engine=inst.engine,
                                sync_info=mybir.SyncInfo(on_wait=[w], on_update=[]),
                                bass_nofuse=True,
                            )
                        )
                    inst.sync_info = mybir.SyncInfo(
                        on_wait=[waits[-1]], on_update=list(si.on_update)
                    )
                out.append(inst)
            bb.instructions = out
    return n


def _build_nc(L_=L, order="interleave_lag", ps_width=1536, ps_bufs=2, pro_ps=False, ld_bufs=2, attn_bufs=3, exp_bufs=3, trn_bufs=1, act_groups=0, v_inter=False, n_pre_groups=2, split_waits=True):
    import concourse.bass as bass
    import concourse.tile as tile
    from concourse import mybir
    from concourse.masks import make_identity
    from concourse.bass import ts, ds

    f32 = mybir.dt.float32
    f32r = mybir.dt.float32r
    bf16 = mybir.dt.bfloat16
    EXP = mybir.ActivationFunctionType.Exp
    LN = mybir.ActivationFunctionType.Ln

    nT = L_ // P  # lq tiles per head
    nG = L_ // 512  # 512-wide lq groups
    nC = L_ // P  # lk chunks

    nc = bass.Bass(
        "TRN2",
        target_bir_lowering=False,
        debug=False,
        enable_asserts=False,
        num_devices=NCORES,
    )

    # q/k/v arrive pre-transposed from the host: [D, L] row-major
    q_d = nc.dram_tensor("qt", [D, L_], f32r, kind="ExternalInput").ap()
    k_d = nc.dram_tensor("kt", [D, L_], f32r, kind="ExternalInput").ap()
    v_d = nc.dram_tensor("vt", [D, L_], f32r, kind="ExternalInput").ap()
    wqt_d = nc.dram_tensor("wqt", [D, HPC * DK], f32r, kind="ExternalInput").ap()
    wkt_d = nc.dram_tensor("wkt", [D, HPC * DK], f32r, kind="ExternalInput").ap()
    wvt_d = nc.dram_tensor("wvt", [D, HPC * DV], f32r, kind="ExternalInput").ap()
    bq_d = nc.dram_tensor("bq", [HPC * DK, 1], f32, kind="ExternalInput").ap()
    bk_d = nc.dram_tensor("bk", [HPC * DK, 1], f32, kind="ExternalInput").ap()
    bv_d = nc.dram_tensor("bv", [1, HPC * DV], f32r, kind="ExternalInput").ap()
    fcwt_d = nc.dram_tensor("fcwt", [HPC * DV, DV], f32r, kind="ExternalInput").ap()

    attn_d = nc.dram_tensor("attn", [HPC, L_, L_], f32, kind="ExternalOutput").ap()
    outp_d = nc.dram_tensor("outp", [HPC, DV, L_], f32, kind="ExternalOutput").ap()

    with tile.TileContext(nc) as tc, ExitStack() as ctx:
        sync, vec, act, pe = nc.sync, nc.vector, nc.scalar, nc.tensor

        const = ctx.enter_context(tc.tile_pool(name="const", bufs=1))
        main = ctx.enter_context(tc.tile_pool(name="main", bufs=1))
        ld = ctx.enter_context(tc.tile_pool(name="ld", bufs=ld_bufs))
        trn = ctx.enter_context(tc.tile_pool(name="trn", bufs=trn_bufs))
        attnp = ctx.enter_context(tc.tile_pool(name="attnp", bufs=attn_bufs))
        expp = ctx.enter_context(tc.tile_pool(name="expp", bufs=exp_bufs))
        stat = ctx.enter_context(tc.tile_pool(name="stat", bufs=8))
        ps = ctx.enter_context(tc.tile_pool(name="ps", bufs=ps_bufs, space="PSUM"))
        misc = ctx.enter_context(tc.tile_pool(name="misc", bufs=2, space="PSUM"))

        ident = const.tile([P, P], f32, tag="ident")
        make_identity(nc, ident)
        ones1 = const.tile([1, P], f32r, tag="ones1")
        vec.memset(ones1.bitcast(f32), 1.0)
        ones1b = const.tile([1, P], bf16, tag="ones1b")
        vec.memset(ones1b, 1.0)

        # --- weights to SBUF ---
        wq_sb = const.tile([P, 4, HPC * DK], f32r, tag="wq")
        sync.dma_start(out=wq_sb, in_=wqt_d.rearrange("(a p) h -> p a h", p=P))
        wk_sb = const.tile([P, 4, HPC * DK], f32r, tag="wk")
        sync.dma_start(out=wk_sb, in_=wkt_d.rearrange("(a p) h -> p a h", p=P))
        wv_sb = const.tile([P, 4, HPC * DV], f32r, tag="wv")
        sync.dma_start(out=wv_sb, in_=wvt_d.rearrange("(a p) h -> p a h", p=P))
        bq_sb = const.tile([DK, HPC], f32, tag="bq")
        sync.dma_start(out=bq_sb, in_=bq_d.rearrange("(h a) x -> a (h x)", h=HPC))
        bk_sb = const.tile([DK, HPC], f32, tag="bk")
        sync.dma_start(out=bk_sb, in_=bk_d.rearrange("(h a) x -> a (h x)", h=HPC))
        bv_sb = const.tile([1, HPC * DV], f32r, tag="bv")
        sync.dma_start(out=bv_sb, in_=bv_d)
        fcw_sb = const.tile([DV, HPC, DV], f32r, tag="fcw")
        sync.dma_start(out=fcw_sb, in_=fcwt_d.rearrange("(h a) d -> a h d", h=HPC))
        wv_sbb = const.tile([P, 4, HPC * DV], bf16, tag="wvb")
        vec.tensor_copy(out=wv_sbb, in_=wv_sb.bitcast(f32))
        bv_sbb = const.tile([1, HPC * DV], bf16, tag="bvb")
        vec.tensor_copy(out=bv_sbb, in_=bv_sb.bitcast(f32))

        # --- persistent per-head tensors ---
        # QTx[h]/KTx[h]: [64, L] Q^T / K^T (temp pre-folded into Q weights).
        # V_sb: [lk-part, chunk, head*(DV+1)] -- per head [V_h | ones]; the
        # ones column makes the ctx matmul accumulate S = rowsum(exp(scores))
        # in psum row 64.
        QTx = [main.tile([DK, L_], f32r, tag=f"qtx{h}", name=f"qtx{h}") for h in range(HPC)]
        KTx = [main.tile([DK, L_], f32r, tag=f"ktx{h}", name=f"ktx{h}") for h in range(HPC)]
        QTb = [main.tile([DK, L_], bf16, tag=f"qtb{h}", name=f"qtb{h}") for h in range(HPC)]
        KTb = [main.tile([DK, L_], bf16, tag=f"ktb{h}", name=f"ktb{h}") for h in range(HPC)]
        V_sb = main.tile([P, nC, HPC * (DV + 2)], bf16, tag="vsb")
        vec.memset(V_sb, 1.0)
        cstage = ctx.enter_context(tc.tile_pool(name="cstage", bufs=2))
        ostage = ctx.enter_context(tc.tile_pool(name="ostage", bufs=2))

        # [D, L] -> [p = d%128, dc = d//128, lq]
        q_r = q_d.rearrange("(a p) l -> p a l", p=P)
        k_r = k_d.rearrange("(a p) l -> p a l", p=P)
        v_r = v_d.rearrange("(a p) l -> p a l", p=P)

        def load_transpose(src_r, g, use_act=False):
            """Load the transposed-layout group [128, 4, 512]: partition = d
            within chunk dc, free = lq in the 512-wide group g."""
            gw = min(4, nT - 4 * g)
            tg = trn.tile([P, 4, 512], f32r, tag="trn")
            sync.dma_start(
                out=tg[:, :, : gw * P], in_=src_r[:, :, ds(g * 512, gw * P)]
            )
            return tg, gw

        # --- prologue: interleave K/Q groups (N-side tile t only needs the
        # K groups its fills touch + Q group t//4), then V ---
        def proj_group(src_r, g, w_sb, b_sb, dst, dstb=None, use_act=False):
            tg, gw = load_transpose(src_r, g, use_act=use_act)
            for h in range(HPC):
                pp = misc.tile([DK, 512], f32, tag="misc")
                for dc in range(4):
                    pe.matmul(
                        pp[:, : gw * P],
                        lhsT=w_sb[:, dc, h * DK : (h + 1) * DK],
                        rhs=tg[:, dc, : gw * P],
                        start=(dc == 0),
                        stop=(dc == 3),
                    )
                if use_act:
                    act.add(
                        out=dst[h][0:DK, ds(g * 512, gw * P)],
                        in_=pp[:, : gw * P],
                        add=b_sb[:, h : h + 1],
                    )
                else:
                    vec.tensor_scalar_add(
                        out=dst[h][0:DK, ds(g * 512, gw * P)],
                        in0=pp[:, : gw * P],
                        scalar1=b_sb[:, h : h + 1],
                    )
                if dstb is not None:
                    vec.tensor_copy(
                        out=dstb[h][0:DK, ds(g * 512, gw * P)],
                        in_=dst[h][0:DK, ds(g * 512, gw * P)].bitcast(f32),
                    )

        def v_group(g):
            tg, gw = load_transpose(v_r, g)
            tgb = trn.tile([P, 4, 512], bf16, tag="trnb")
            vec.tensor_copy(out=tgb[:, :, : gw * P], in_=tg[:, :, : gw * P].bitcast(f32))
            for tt in range(gw):
                c = 4 * g + tt
                pv = (ps if pro_ps else misc).tile([P, HPC * DV], f32, tag="ps" if pro_ps else "misc", name="pv")
                for dc in range(4):
                    pe.matmul(
                        pv,
                        lhsT=tgb[:, dc, ts(tt, P)],
                        rhs=wv_sbb[:, dc, :],
                        start=(dc == 0),
                        stop=False,
                    )
                pe.matmul(
                    pv,
                    lhsT=ones1b,
                    rhs=bv_sbb,
                    start=False,
                    stop=True,
                )
                for h in range(HPC):
                    vec.tensor_copy(
                        out=V_sb[:, c, h * (DV + 2) : h * (DV + 2) + DV],
                        in_=pv[:, h * DV : (h + 1) * DV],
                    )

        for g in range((nT + 3) // 4):
            proj_group(k_r, g, wk_sb, bk_sb, KTx, dstb=KTb, use_act=(g < act_groups))
            proj_group(q_r, g, wq_sb, bq_sb, QTx, dstb=QTb, use_act=(g < act_groups))
            if v_inter:
                v_group(g)
        if not v_inter:
            for g in range((nT + 3) // 4):
                v_group(g)

        n_fills = _fills(L_, ps_width)
        t_groups = _fills(nC, ps_width // 512)

        def emit_t_group(h, g):
            """T-side: scoresT = KT.T @ QT (raw), exp, ctx^T + S via the ones
            column, then normalize ctx^T, fc, and produce -lnS columns for the
            N-side tiles of this group."""
            gw4 = min(4, nT - 4 * g)
            pc = misc.tile([DV + 2, 512], f32, tag="misc", name="pc")
            for cbase, nch in t_groups:
                pt = ps.tile([P, nch * 512], f32, tag="ps", name="pt")
                et = expp.tile([P, nch * 512], bf16, tag="expp", name="et")
                for cc in range(nch):
                    c = cbase + cc
                    pe.matmul(
                        pt[:, ts(cc, 512)],
                        lhsT=KTb[h][:, ts(c, P)],
                        rhs=QTb[h][:, ts(g, 512)],
                        start=True,
                        stop=True,
                    )
                act.activation(out=et, in_=pt, func=EXP)
                for cc in range(nch):
                    c = cbase + cc
                    pe.matmul(
                        pc,
                        lhsT=V_sb[:, c, h * (DV + 2) : (h + 1) * (DV + 2)],
                        rhs=et[:, ts(cc, 512)],
                        start=(c == 0),
                        stop=(c == nC - 1),
                    )
            # stage ctx^T + S to SBUF; derive invS (transposed), -lnS, and the
            # broadcast row for normalization
            cs = cstage.tile([DV + 1, 512], f32, tag="cstage", name="cs")
            vec.tensor_copy(out=cs, in_=pc[0 : DV + 1, :])
            pst = misc.tile([P, 4], f32, tag="misc", name="pst")
            for j in range(gw4):
                pe.transpose(
                    out=pst[:, j : j + 1],
                    in_=cs[DV : DV + 1, ts(j, P)],
                    identity=ident[DV : DV + 1, DV : DV + 1],
                )
            invS_t = stat.tile([P, 4], f32, tag="invs", bufs=4)
            vec.reciprocal(out=invS_t[:, :gw4], in_=pst[:, :gw4])
            nlnS = stat.tile([P, 4], f32, tag="nlns", bufs=4)
            act.activation(out=nlnS[:, :gw4], in_=invS_t[:, :gw4], func=LN)
            prow = misc.tile([1, 512], f32, tag="misc", name="prow")
            for j in range(gw4):
                pe.transpose(
                    out=prow[0:1, ts(j, P)], in_=invS_t[:, j : j + 1], identity=ident
                )
            invS_row = stat.tile([1, 512], f32r, tag="invsrow", bufs=2)
            vec.tensor_copy(out=invS_row[0:1, : gw4 * P], in_=prow[0:1, : gw4 * P])
            pb = misc.tile([P, 512], f32, tag="misc", name="pb")
            pe.matmul(
                pb[:, : gw4 * P],
                lhsT=ones1,
                rhs=invS_row[0:1, : gw4 * P],
                start=True,
                stop=True,
            )
            cnorm = cstage.tile([DV, 512], f32r, tag="cstage", name="cnorm")
            vec.tensor_mul(cnorm[:, : gw4 * P], cs[0:DV, : gw4 * P], pb[0:DV, : gw4 * P])
            # fc: out^T = fcw_h.T @ ctx^T
            pf = misc.tile([DV, 512], f32, tag="misc", name="pf")
            pe.matmul(
                pf[:, : gw4 * P],
                lhsT=fcw_sb[:, h, :],
                rhs=cnorm[:, : gw4 * P],
                start=True,
                stop=True,
            )
            osb = ostage.tile([DV, 512], f32, tag="ostage", name="osb")
            vec.tensor_copy(out=osb[:, : gw4 * P], in_=pf[:, : gw4 * P])
            sync.dma_start(out=outp_d[h, :, ds(g * 512, gw4 * P)], in_=osb[:, : gw4 * P])
            return nlnS

        def emit_n_tile_v1(h, t):
            """Self-sufficient N-side tile (no T-group dependency): raw exp
            with fused row-sum, then DVE normalize. Used for the first group
            so ACT has work while the V prologue finishes."""
            at = attnp.tile([P, L_], f32, tag="attn", name="at")
            sp = stat.tile([P, 8], f32, tag="stat", bufs=4)
            for f, (base, fw) in enumerate(n_fills):
                pn = ps.tile([P, fw], f32, tag="ps", name="pn")
                for jj in range(fw // 512):
                    pe.matmul(
                        pn[:, ts(jj, 512)],
                        lhsT=QTx[h][:, ts(t, P)],
                        rhs=KTx[h][:, ds(base + jj * 512, 512)],
                        start=True,
                        stop=True,
                    )
                act.activation(
                    out=at[:, ds(base, fw)],
                    in_=pn,
                    func=EXP,
                    accum_out=sp[:, f : f + 1],
                )
            nf = len(n_fills)
            if nf == 1:
                vec.reciprocal(sp[:, 5:6], sp[:, 0:1])
            else:
                vec.tensor_add(sp[:, 4:5], sp[:, 0:1], sp[:, 1:2])
                for f in range(2, nf):
                    vec.tensor_add(sp[:, 4:5], sp[:, 4:5], sp[:, f : f + 1])
                vec.reciprocal(sp[:, 5:6], sp[:, 4:5])
            vec.tensor_scalar_mul(out=at, in0=at, scalar1=sp[:, 5:6])
            sync.dma_start(out=attn_d[h, ts(t, P), :], in_=at)

        def emit_n_tile(h, t, nlnS):
            """N-side: natural scores, then exp(scores - lnS) via the ACT
            per-partition bias -> normalized attention rows, straight to HBM."""
            j = t % 4
            at = attnp.tile([P, L_], f32, tag="attn", name="at")
            for base, fw in n_fills:
                pn = ps.tile([P, fw], f32, tag="ps", name="pn")
                for jj in range(fw // 512):
                    pe.matmul(
                        pn[:, ts(jj, 512)],
                        lhsT=QTx[h][:, ts(t, P)],
                        rhs=KTx[h][:, ds(base + jj * 512, 512)],
                        start=True,
                        stop=True,
                    )
                act.activation(
                    out=at[:, ds(base, fw)],
                    in_=pn,
                    func=EXP,
                    bias=nlnS[:, j : j + 1],
                )
            sync.dma_start(out=attn_d[h, ts(t, P), :], in_=at)

        # First group's N tiles are self-sufficient (v1 style) so ACT has
        # work while V finishes; afterwards T group first (produces -lnS),
        # then its four bias-normalized N tiles.
        n_pre = min(n_pre_groups, nG)
        for h in range(HPC):
            for t in range(min(4 * n_pre, nT)):
                emit_n_tile_v1(h, t)
        for g in range(nG):
            for h in range(HPC):
                nlnS = emit_t_group(h, g)
                if g >= n_pre:
                    for t in range(4 * g, min(4 * g + 4, nT)):
                        emit_n_tile(h, t, nlnS)

    if split_waits:
        _split_multi_waits(nc)
    return nc


def _shard_inputs(inputs):
    """Full inputs -> per-core input maps (host-side, cheap)."""
    q, k, v = inputs["q"], inputs["k"], inputs["v"]
    wq, bq, wk, bk = inputs["wq"], inputs["bq"], inputs["wk"], inputs["bk"]
    wv, bv, fc_w = inputs["wv"], inputs["bv"], inputs["fc_w"]
    f32 = np.float32
    # pre-transpose per batch once (shared by the 4 cores of that batch)
    qT = [np.ascontiguousarray(q[b].T, dtype=f32) for b in range(B)]
    kT = [np.ascontiguousarray(k[b].T, dtype=f32) for b in range(B)]
    vT = [np.ascontiguousarray(v[b].T, dtype=f32) for b in range(B)]
    in_maps = []
    for d in range(NCORES):
        b = d // 4
        h0 = HPC * (d % 4)
        rows = slice(h0 * DK, (h0 + HPC) * DK)
        in_maps.append(
            {
                "qt": qT[b],
                "kt": kT[b],
                "vt": vT[b],
                "wqt": np.ascontiguousarray((wq[rows] / TEMP).T, dtype=f32),
                "wkt": np.ascontiguousarray(wk[rows].T, dtype=f32),
                "wvt": np.ascontiguousarray(wv[rows].T, dtype=f32),
                "bq": np.ascontiguousarray(
                    (bq[rows] / TEMP).reshape(HPC * DK, 1), dtype=f32
                ),
                "bk": np.ascontiguousarray(bk[rows].reshape(HPC * DK, 1), dtype=f32),
                "bv": np.ascontiguousarray(bv[rows].reshape(1, HPC * DV), dtype=f32),
                "fcwt": np.ascontiguousarray(fc_w[:, rows].T, dtype=f32),
            }
        )
    return in_maps


def _get_runner():
    """Build the Bass module once and return a cached jitted SPMD callable."""
    if "runner" in _RUNNER_CACHE:
        return _RUNNER_CACHE["runner"]

    import jax
    from jax.sharding import Mesh, PartitionSpec, NamedSharding

    try:
        from jax.experimental.shard_map import shard_map
    except ImportError:  # newer jax
        shard_map = jax.shard_map
    from concourse import bass2jax, mybir

    bass2jax.install_neuronx_cc_hook()
    nc = _build_nc()

    partition_name = nc.partition_id_tensor.name if nc.partition_id_tensor else None
    in_names, out_names, out_avals, out_shapes = [], [], [], []
    for alloc in nc.m.functions[0].allocations:
        if not isinstance(alloc, mybir.MemoryLocationSet):
            continue
        if alloc.kind not in ("ExternalInput", "ExternalOutput"):
            continue
        name = alloc.memorylocations[0].name
        if alloc.kind == "ExternalInput":
            if name != partition_name:
                in_names.append(name)
        else:
            out_names.append(name)
            shape = tuple(alloc.tensor_shape)
            dtype = mybir.dt.np(alloc.dtype)
            out_avals.append(jax.core.ShapedArray(shape, dtype))
            out_shapes.append((shape, dtype))
    n_params = len(in_names)
    all_in_names = list(in_names) + list(out_names)
    if partition_name is not None:
        all_in_names.append(partition_name)
    all_in_names = tuple(all_in_names)
    donate = tuple(range(n_params, n_params + len(out_names)))

    def _body(*args):
        operands = list(args)
        if partition_name is not None:
            operands.append(bass2jax.partition_id_tensor())
        outs = bass2jax._bass_exec_p.bind(
            *operands,
            out_avals=tuple(out_avals),
            in_names=all_in_names,
            out_names=tuple(out_names),
            lowering_input_output_aliases=(),
            sim_require_finite=True,
            sim_require_nnan=True,
            nc=nc,
        )
        return tuple(outs)

    devices = jax.devices()[:NCORES]
    assert len(devices) == NCORES, f"need {NCORES} cores, got {len(jax.devices())}"
    mesh = Mesh(np.asarray(devices), ("core",))
    in_specs = (PartitionSpec("core"),) * (n_params + len(out_names))
    out_specs = (PartitionSpec("core"),) * len(out_names)
    fn = jax.jit(
        shard_map(
            _body, mesh=mesh, in_specs=in_specs, out_specs=out_specs, check_rep=False
        ),
        donate_argnums=donate,
        keep_unused=True,
    )
    sharding = NamedSharding(mesh, PartitionSpec("core"))

    # on-device zero buffers for the donated outputs (kernel writes every
    # element, but the custom-call path wants donated operands to reuse)
    def _zeros():
        import jax.numpy as jnp

        return tuple(
            jnp.zeros((NCORES * s[0],) + tuple(s[1:]), dt) for (s, dt) in out_shapes
        )

    zeros_fn = jax.jit(_zeros, out_shardings=(sharding,) * len(out_shapes))

    runner = (jax, fn, zeros_fn, in_names, out_names, sharding)
    _RUNNER_CACHE["runner"] = runner
    return runner


def kernel(**inputs):
    global LAST_EXEC_S
    jax, fn, zeros_fn, in_names, out_names, sharding = _get_runner()

    in_maps = _shard_inputs(inputs)
    concat = [
        jax.device_put(
            np.concatenate([in_maps[c][nm] for c in range(NCORES)], axis=0), sharding
        )
        for nm in in_names
    ]
    zouts = zeros_fn()
    jax.block_until_ready((concat, zouts))

    t0 = time.perf_counter()
    outs = fn(*concat, *zouts)
    outs = jax.block_until_ready(outs)
    LAST_EXEC_S = time.perf_counter() - t0

    res = {nm: np.asarray(o) for nm, o in zip(out_names, outs)}
    # attn: [8*2, L, L]; core d rows [2d, 2d+1] are heads (2*(d%4), 2*(d%4)+1)
    # of batch d//4. attn_view[h*B + b] = global row 8*b + h.
    ga = res["attn"]
    idx = [8 * b + h for h in range(H) for b in range(B)]
    attn_view = ga[idx]
    # outp: per-core [HPC, DV, L] partial outT per head
    go = res["outp"].reshape(NCORES, HPC, DV, L)
    fc_b = np.asarray(inputs["fc_b"], dtype=np.float32)
    out = np.empty((B, L, DV), dtype=np.float32)
    for b in range(B):
        acc = go[4 * b : 4 * b + 4].sum(axis=(0, 1))  # [DV, L]
        out[b] = acc.T + fc_b
    return out, attn_view


# revision 42
# speedup vs baseline: 1.4575x; 1.3852x over previous
"""Trainium2 Bass kernel for nn_MultiHeadAttention_5308579578426.

Multi-head attention, B=2 L=4096 D=512 H=8 DK=DV=64, returning both the
projected output [B, L, DV] and the full attention matrix [H*B, L, L].

Sharding (data + head parallel, no cross-device comm):
  core d -> batch b = d // 4, heads {2*(d%4), 2*(d%4)+1}.
Each core computes its two heads' full attention rows (written straight to
HBM) and a partial fc output (summed over its heads); the host sums the
per-batch partials and adds fc_b.

Per-core algorithm (all fp32; matmuls use the full-rate float32r PE path):
  prologue: load q/k/v [L, D], transpose via PE into [d, l] layout, project
            into QT/KT [head][65, L] and V [lk, hv] with biases folded in.
  N-side  : per (head, 128-row lq tile): scores = QT.T @ KT (K=64 matmuls),
            exp via ACT with fused row-sum accumulation, normalize rows via
            DVE tensor_scalar, DMA the [128, L] attention tile to HBM.
            Also writes row 64 of QT with -ln(rowsum) (PE-transposed).
  T-side  : per (head, 512-col lq group): scoresT = KTx.T @ QTx with K=65 --
            the 65th row pair (ones, -lnS) makes the matmul emit
            scoresT - lnS, so the ACT exp directly yields normalized
            attention^T, which feeds ctx^T += V_chunk.T @ attnT_chunk.
  fc      : outT = fcw.T @ ctxT accumulated over both heads, DMA to HBM.
"""

import os
import time
from contextlib import ExitStack

import numpy as np

B, L, D, H, DK, DV = 2, 4096, 512, 8, 64, 64
P = 128
NCORES = 8
HPC = 2  # heads per core
TEMP = 8.0  # sqrt(DK)

_RUNNER_CACHE = {}
LAST_EXEC_S = None  # wall time of the device execution of the last kernel() call


def _fills(total, width):
    """Split `total` into chunks of `width` (last chunk may be smaller)."""
    out = []
    base = 0
    while base < total:
        w = min(width, total - base)
        out.append((base, w))
        base += w
    return out


def _split_multi_waits(nc):
    """The container's walrus accepts at most one sync-wait per instruction;
    Tile attaches several. Hoist all but the last wait of each instruction
    onto no-op instructions inserted immediately before it (same engine, same
    program point -- semantically identical, sequencer waits serially)."""
    from concourse import mybir

    n = 0
    for fn in nc.m.functions:
        for bb in fn.blocks:
            insts = list(bb.instructions)
            if not any(
                i.sync_info is not None and len(i.sync_info.on_wait) > 1
                for i in insts
            ):
                continue
            out = []
            for inst in insts:
                si = inst.sync_info
                if si is not None and len(si.on_wait) > 1:
                    waits = list(si.on_wait)
                    for w in waits[:-1]:
                        n += 1
                        out.append(
                            mybir.InstNoOp(
                                name=f"WSPLIT-{nc.next_id()}",
                                engine=inst.engine,
                                sync_info=mybir.SyncInfo(on_wait=[w], on_update=[]),
                                bass_nofuse=True,
                            )
                        )
                    inst.sync_info = mybir.SyncInfo(
                        on_wait=[waits[-1]], on_update=list(si.on_update)
                    )
                out.append(inst)
            bb.instructions = out
    return n


def _build_nc(L_=L, ld_bufs=2, attn_bufs=4, exp_bufs=3, trn_bufs=2,
              act_groups=0, v_inter=False, n_pre_groups=2, split_waits=True):
    import concourse.bass as bass
    import concourse.tile as tile
    from concourse import mybir
    from concourse.masks import make_identity
    from concourse.bass import ts, ds

    f32 = mybir.dt.float32
    f32r = mybir.dt.float32r
    f16 = mybir.dt.float16
    EXP = mybir.ActivationFunctionType.Exp
    LN = mybir.ActivationFunctionType.Ln

    assert L_ % 1024 == 0
    nT = L_ // P  # lq tiles per head
    nG = L_ // 512  # 512-wide lq groups
    nC = L_ // P  # lk chunks
    nE = nG // 2  # 1024-wide bank pairs

    nc = bass.Bass(
        "TRN2",
        target_bir_lowering=False,
        debug=False,
        enable_asserts=False,
        num_devices=NCORES,
    )

    # q/k/v arrive pre-transposed from the host: [D, L] row-major
    q_d = nc.dram_tensor("qt", [D, L_], f32r, kind="ExternalInput").ap()
    k_d = nc.dram_tensor("kt", [D, L_], f32r, kind="ExternalInput").ap()
    v_d = nc.dram_tensor("vt", [D, L_], f32r, kind="ExternalInput").ap()
    wqt_d = nc.dram_tensor("wqt", [D, HPC * DK], f32r, kind="ExternalInput").ap()
    wkt_d = nc.dram_tensor("wkt", [D, HPC * DK], f32r, kind="ExternalInput").ap()
    wvt_d = nc.dram_tensor("wvt", [D, HPC * DV], f32r, kind="ExternalInput").ap()
    bq_d = nc.dram_tensor("bq", [HPC * DK, 1], f32, kind="ExternalInput").ap()
    bk_d = nc.dram_tensor("bk", [HPC * DK, 1], f32, kind="ExternalInput").ap()
    bv_d = nc.dram_tensor("bv", [1, HPC * DV], f32r, kind="ExternalInput").ap()
    fcwt_d = nc.dram_tensor("fcwt", [HPC * DV, DV], f32r, kind="ExternalInput").ap()

    attn_d = nc.dram_tensor("attn", [HPC, L_, L_], f32, kind="ExternalOutput").ap()
    outp_d = nc.dram_tensor("outp", [HPC, DV, L_], f32, kind="ExternalOutput").ap()

    with tile.TileContext(nc) as tc, ExitStack() as ctx:
        sync, vec, act, pe = nc.sync, nc.vector, nc.scalar, nc.tensor

        const = ctx.enter_context(tc.tile_pool(name="const", bufs=1))
        main = ctx.enter_context(tc.tile_pool(name="main", bufs=1))
        trn = ctx.enter_context(tc.tile_pool(name="trn", bufs=trn_bufs))
        attnp = ctx.enter_context(tc.tile_pool(name="attnp", bufs=attn_bufs))
        expp = ctx.enter_context(tc.tile_pool(name="expp", bufs=exp_bufs))
        stat = ctx.enter_context(tc.tile_pool(name="stat", bufs=8))
        ps = ctx.enter_context(tc.tile_pool(name="ps", bufs=3, space="PSUM"))
        misc = ctx.enter_context(tc.tile_pool(name="misc", bufs=2, space="PSUM"))

        ident = const.tile([P, P], f32, tag="ident")
        make_identity(nc, ident)
        ones1 = const.tile([1, P], f32r, tag="ones1")
        vec.memset(ones1.bitcast(f32), 1.0)
        ones1h = const.tile([1, P], f16, tag="ones1h")
        vec.memset(ones1h, 1.0)

        # --- weights to SBUF ---
        wq_sb = const.tile([P, 4, HPC * DK], f32r, tag="wq")
        sync.dma_start(out=wq_sb, in_=wqt_d.rearrange("(a p) h -> p a h", p=P))
        wk_sb = const.tile([P, 4, HPC * DK], f32r, tag="wk")
        sync.dma_start(out=wk_sb, in_=wkt_d.rearrange("(a p) h -> p a h", p=P))
        wv_sb = const.tile([P, 4, HPC * DV], f32r, tag="wv")
        sync.dma_start(out=wv_sb, in_=wvt_d.rearrange("(a p) h -> p a h", p=P))
        bq_sb = const.tile([DK, HPC], f32, tag="bq")
        sync.dma_start(out=bq_sb, in_=bq_d.rearrange("(h a) x -> a (h x)", h=HPC))
        bk_sb = const.tile([DK, HPC], f32, tag="bk")
        sync.dma_start(out=bk_sb, in_=bk_d.rearrange("(h a) x -> a (h x)", h=HPC))
        bv_sb = const.tile([1, HPC * DV], f32r, tag="bv")
        sync.dma_start(out=bv_sb, in_=bv_d)
        fcw_sb = const.tile([DV, HPC, DV], f32r, tag="fcw")
        sync.dma_start(out=fcw_sb, in_=fcwt_d.rearrange("(h a) d -> a h d", h=HPC))
        wv_sbh = const.tile([P, 4, HPC * DV], f16, tag="wvh")
        vec.tensor_copy(out=wv_sbh, in_=wv_sb.bitcast(f32))
        bv_sbh = const.tile([1, HPC * DV], f16, tag="bvh")
        vec.tensor_copy(out=bv_sbh, in_=bv_sb.bitcast(f32))

        # --- persistent per-head tensors (fp16, row-pack friendly) ---
        # QTd[h]: [128, L]; rows 0..63 = Q^T, rows 64..127 = a copy, so the
        #   idle half of the PE array can run a second K=64 matmul.
        # KTi[h]: [128, L/2]; rows 0..63 hold the EVEN 512-wide lk banks,
        #   rows 64..127 the ODD banks (bank pair e at free ts(e, 512)).
        # V_sb: [lk-part, chunk, head*(DV+2)] -- per head [V | ones | ones];
        #   the ones column makes the ctx matmul accumulate S in psum row 64.
        QTd = [main.tile([P, L_], f16, tag=f"qtd{h}", name=f"qtd{h}") for h in range(HPC)]
        KTi = [main.tile([P, L_ // 2], f16, tag=f"kti{h}", name=f"kti{h}") for h in range(HPC)]
        V_sb = main.tile([P, nC, HPC * (DV + 2)], f16, tag="vsb")
        vec.memset(V_sb, 1.0)
        cstage = ctx.enter_context(tc.tile_pool(name="cstage", bufs=2))
        ostage = ctx.enter_context(tc.tile_pool(name="ostage", bufs=2))

        # [D, L] -> [p = d%128, dc = d//128, lq]
        q_r = q_d.rearrange("(a p) l -> p a l", p=P)
        k_r = k_d.rearrange("(a p) l -> p a l", p=P)
        v_r = v_d.rearrange("(a p) l -> p a l", p=P)

        def load_group(src_r, g):
            """Load the transposed-layout group [128, 4, 512]: partition = d
            within chunk dc, free = lq in the 512-wide group g."""
            gw = min(4, nT - 4 * g)
            tg = trn.tile([P, 4, 512], f32r, tag="trn")
            sync.dma_start(
                out=tg[:, :, : gw * P], in_=src_r[:, :, ds(g * 512, gw * P)]
            )
            return tg, gw

        def proj_group(src_r, g, w_sb, b_sb, kind):
            """Project one 512-wide lq/lk group for both heads into the fp16
            packed layouts (kind='q' -> QTd + dup, kind='k' -> KTi)."""
            tg, gw = load_group(src_r, g)
            for h in range(HPC):
                pp = misc.tile([DK, 512], f32, tag="misc")
                for dc in range(4):
                    pe.matmul(
                        pp[:, : gw * P],
                        lhsT=w_sb[:, dc, h * DK : (h + 1) * DK],
                        rhs=tg[:, dc, : gw * P],
                        start=(dc == 0),
                        stop=(dc == 3),
                    )
                if kind == "q":
                    vec.tensor_scalar_add(
                        out=QTd[h][0:DK, ds(g * 512, gw * P)],
                        in0=pp[:, : gw * P],
                        scalar1=b_sb[:, h : h + 1],
                    )
                    # duplicate into rows 64..127 (cross-partition -> DMA)
                    sync.dma_start(
                        out=QTd[h][DK:P, ds(g * 512, gw * P)],
                        in_=QTd[h][0:DK, ds(g * 512, gw * P)],
                    )
                else:
                    vec.tensor_scalar_add(
                        out=KTi[h][
                            (g % 2) * DK : (g % 2) * DK + DK,
                            ds((g // 2) * 512, gw * P),
                        ],
                        in0=pp[:, : gw * P],
                        scalar1=b_sb[:, h : h + 1],
                    )

        def v_group(g):
            tg, gw = load_group(v_r, g)
            tgh = trn.tile([P, 4, 512], f16, tag="trnh")
            vec.tensor_copy(out=tgh[:, :, : gw * P], in_=tg[:, :, : gw * P].bitcast(f32))
            for tt in range(gw):
                c = 4 * g + tt
                pv = misc.tile([P, HPC * DV], f32, tag="misc", name="pv")
                for dc in range(4):
                    pe.matmul(
                        pv,
                        lhsT=tgh[:, dc, ts(tt, P)],
                        rhs=wv_sbh[:, dc, :],
                        start=(dc == 0),
                        stop=False,
                    )
                pe.matmul(
                    pv,
                    lhsT=ones1h,
                    rhs=bv_sbh,
                    start=False,
                    stop=True,
                )
                for h in range(HPC):
                    vec.tensor_copy(
                        out=V_sb[:, c, h * (DV + 2) : h * (DV + 2) + DV],
                        in_=pv[:, h * DV : (h + 1) * DV],
                    )

        for g in range((nT + 3) // 4):
            proj_group(k_r, g, wk_sb, bk_sb, "k")
            proj_group(q_r, g, wq_sb, bq_sb, "q")
            if v_inter:
                v_group(g)
        if not v_inter:
            for g in range((nT + 3) // 4):
                v_group(g)

        def scores_nat_fill(h, t, e, pn):
            """Packed pair of natural-score matmuls: lk banks 2e (rows 0-63)
            and 2e+1 (rows 64-127) -> pn[:, 0:512] / pn[:, 512:1024]."""
            pe.matmul(
                pn[:, 0:512],
                lhsT=QTd[h][0:DK, ts(t, P)],
                rhs=KTi[h][0:DK, ts(e, 512)],
                start=True,
                stop=True,
            )
            pe.matmul(
                pn[:, 512:1024],
                lhsT=QTd[h][DK:P, ts(t, P)],
                rhs=KTi[h][DK:P, ts(e, 512)],
                start=True,
                stop=True,
            )

        def emit_t_group(h, g):
            """T-side: packed scoresT pairs, exp, ctx^T + S via the ones
            column, normalize ctx^T, fc, and -lnS columns for the N-side."""
            pc = misc.tile([DV + 2, 512], f32, tag="misc", name="pc")
            first = True
            for e in range(nE):
                for i in range(4):
                    cA = 8 * e + i        # chunk in even bank 2e
                    cB = 8 * e + 4 + i    # chunk in odd bank 2e+1
                    pt = ps.tile([P, 1024], f32, tag="ps", name="pt")
                    et = expp.tile([P, 1024], f16, tag="expp", name="et")
                    pe.matmul(
                        pt[:, 0:512],
                        lhsT=KTi[h][0:DK, ds(e * 512 + i * P, P)],
                        rhs=QTd[h][0:DK, ts(g, 512)],
                        start=True,
                        stop=True,
                    )
                    pe.matmul(
                        pt[:, 512:1024],
                        lhsT=KTi[h][DK:P, ds(e * 512 + i * P, P)],
                        rhs=QTd[h][DK:P, ts(g, 512)],
                        start=True,
                        stop=True,
                    )
                    act.activation(out=et, in_=pt, func=EXP)
                    last = e == nE - 1 and i == 3
                    pe.matmul(
                        pc,
                        lhsT=V_sb[:, cA, h * (DV + 2) : (h + 1) * (DV + 2)],
                        rhs=et[:, 0:512],
                        start=first,
                        stop=False,
                    )
                    pe.matmul(
                        pc,
                        lhsT=V_sb[:, cB, h * (DV + 2) : (h + 1) * (DV + 2)],
                        rhs=et[:, 512:1024],
                        start=False,
                        stop=last,
                    )
                    first = False
            # stage ctx^T + S to SBUF; derive invS (transposed), -lnS, and
            # the broadcast row for normalization
            cs = cstage.tile([DV + 1, 512], f32, tag="cstage", name="cs")
            vec.tensor_copy(out=cs, in_=pc[0 : DV + 1, :])
            pst = misc.tile([P, 4], f32, tag="misc", name="pst")
            for j in range(4):
                pe.transpose(
                    out=pst[:, j : j + 1],
                    in_=cs[DV : DV + 1, ts(j, P)],
                    identity=ident[DV : DV + 1, DV : DV + 1],
                )
            invS_t = stat.tile([P, 4], f32, tag="invs", bufs=4)
            vec.reciprocal(out=invS_t, in_=pst)
            nlnS = stat.tile([P, 4], f32, tag="nlns", bufs=4)
            act.activation(out=nlnS, in_=invS_t, func=LN)
            prow = misc.tile([1, 512], f32, tag="misc", name="prow")
            for j in range(4):
                pe.transpose(
                    out=prow[0:1, ts(j, P)], in_=invS_t[:, j : j + 1], identity=ident
                )
            invS_row = stat.tile([1, 512], f32r, tag="invsrow", bufs=2)
            vec.tensor_copy(out=invS_row, in_=prow)
            pb = misc.tile([P, 512], f32, tag="misc", name="pb")
            pe.matmul(pb, lhsT=ones1, rhs=invS_row, start=True, stop=True)
            cnorm = cstage.tile([DV, 512], f32r, tag="cstage", name="cnorm")
            vec.tensor_mul(cnorm, cs[0:DV, :], pb[0:DV, :])
            # fc: out^T = fcw_h.T @ ctx^T
            pf = misc.tile([DV, 512], f32, tag="misc", name="pf")
            pe.matmul(pf, lhsT=fcw_sb[:, h, :], rhs=cnorm, start=True, stop=True)
            osb = ostage.tile([DV, 512], f32, tag="ostage", name="osb")
            vec.tensor_copy(out=osb, in_=pf)
            sync.dma_start(out=outp_d[h, :, ts(g, 512)], in_=osb)
            return nlnS

        def emit_n_tile_v1(h, t):
            """Self-sufficient N-side tile (no T-group dependency): raw exp
            with fused row-sum, then DVE normalize. Used for the first groups
            so ACT has work while the V prologue finishes."""
            at = attnp.tile([P, L_], f32, tag="attn", name="at")
            sp = stat.tile([P, 8], f32, tag="stat", bufs=4)
            for e in range(nE):
                pn = ps.tile([P, 1024], f32, tag="ps", name="pn")
                scores_nat_fill(h, t, e, pn)
                act.activation(
                    out=at[:, ds(e * 1024, 1024)],
                    in_=pn,
                    func=EXP,
                    accum_out=sp[:, e : e + 1],
                )
            if nE == 1:
                vec.reciprocal(sp[:, 5:6], sp[:, 0:1])
            else:
                vec.tensor_add(sp[:, 4:5], sp[:, 0:1], sp[:, 1:2])
                for e in range(2, nE):
                    vec.tensor_add(sp[:, 4:5], sp[:, 4:5], sp[:, e : e + 1])
                vec.reciprocal(sp[:, 5:6], sp[:, 4:5])
            vec.tensor_scalar_mul(out=at, in0=at, scalar1=sp[:, 5:6])
            sync.dma_start(out=attn_d[h, ts(t, P), :], in_=at)

        def emit_n_tile(h, t, nlnS):
            """N-side: packed natural scores, then exp(scores - lnS) via the
            ACT per-partition bias -> normalized attention rows, to HBM."""
            j = t % 4
            at = attnp.tile([P, L_], f32, tag="attn", name="at")
            for e in range(nE):
                pn = ps.tile([P, 1024], f32, tag="ps", name="pn")
                scores_nat_fill(h, t, e, pn)
                act.activation(
                    out=at[:, ds(e * 1024, 1024)],
                    in_=pn,
                    func=EXP,
                    bias=nlnS[:, j : j + 1],
                )
            sync.dma_start(out=attn_d[h, ts(t, P), :], in_=at)

        # First groups' N tiles are self-sufficient (v1 style) so ACT has
        # work while the V prologue finishes; afterwards T group first
        # (produces -lnS), then its four bias-normalized N tiles.
        n_pre = min(n_pre_groups, nG)
        for h in range(HPC):
            for t in range(min(4 * n_pre, nT)):
                emit_n_tile_v1(h, t)
        for g in range(nG):
            for h in range(HPC):
                nlnS = emit_t_group(h, g)
                if g >= n_pre:
                    for t in range(4 * g, min(4 * g + 4, nT)):
                        emit_n_tile(h, t, nlnS)

    if split_waits:
        _split_multi_waits(nc)
    return nc


def _shard_inputs(inputs):
    """Full inputs -> per-core input maps (host-side, cheap)."""
    q, k, v = inputs["q"], inputs["k"], inputs["v"]
    wq, bq, wk, bk = inputs["wq"], inputs["bq"], inputs["wk"], inputs["bk"]
    wv, bv, fc_w = inputs["wv"], inputs["bv"], inputs["fc_w"]
    f32 = np.float32
    # pre-transpose per batch once (shared by the 4 cores of that batch)
    qT = [np.ascontiguousarray(q[b].T, dtype=f32) for b in range(B)]
    kT = [np.ascontiguousarray(k[b].T, dtype=f32) for b in range(B)]
    vT = [np.ascontiguousarray(v[b].T, dtype=f32) for b in range(B)]
    in_maps = []
    for d in range(NCORES):
        b = d // 4
        h0 = HPC * (d % 4)
        rows = slice(h0 * DK, (h0 + HPC) * DK)
        in_maps.append(
            {
                "qt": qT[b],
                "kt": kT[b],
                "vt": vT[b],
                "wqt": np.ascontiguousarray((wq[rows] / TEMP).T, dtype=f32),
                "wkt": np.ascontiguousarray(wk[rows].T, dtype=f32),
                "wvt": np.ascontiguousarray(wv[rows].T, dtype=f32),
                "bq": np.ascontiguousarray(
                    (bq[rows] / TEMP).reshape(HPC * DK, 1), dtype=f32
                ),
                "bk": np.ascontiguousarray(bk[rows].reshape(HPC * DK, 1), dtype=f32),
                "bv": np.ascontiguousarray(bv[rows].reshape(1, HPC * DV), dtype=f32),
                "fcwt": np.ascontiguousarray(fc_w[:, rows].T, dtype=f32),
            }
        )
    return in_maps


def _get_runner():
    """Build the Bass module once and return a cached jitted SPMD callable."""
    if "runner" in _RUNNER_CACHE:
        return _RUNNER_CACHE["runner"]

    import jax
    from jax.sharding import Mesh, PartitionSpec, NamedSharding

    try:
        from jax.experimental.shard_map import shard_map
    except ImportError:  # newer jax
        shard_map = jax.shard_map
    from concourse import bass2jax, mybir

    bass2jax.install_neuronx_cc_hook()
    nc = _build_nc()

    partition_name = nc.partition_id_tensor.name if nc.partition_id_tensor else None
    in_names, out_names, out_avals, out_shapes = [], [], [], []
    for alloc in nc.m.functions[0].allocations:
        if not isinstance(alloc, mybir.MemoryLocationSet):
            continue
        if alloc.kind not in ("ExternalInput", "ExternalOutput"):
            continue
        name = alloc.memorylocations[0].name
        if alloc.kind == "ExternalInput":
            if name != partition_name:
                in_names.append(name)
        else:
            out_names.append(name)
            shape = tuple(alloc.tensor_shape)
            dtype = mybir.dt.np(alloc.dtype)
            out_avals.append(jax.core.ShapedArray(shape, dtype))
            out_shapes.append((shape, dtype))
    n_params = len(in_names)
    all_in_names = list(in_names) + list(out_names)
    if partition_name is not None:
        all_in_names.append(partition_name)
    all_in_names = tuple(all_in_names)
    donate = tuple(range(n_params, n_params + len(out_names)))

    def _body(*args):
        operands = list(args)
        if partition_name is not None:
            operands.append(bass2jax.partition_id_tensor())
        outs = bass2jax._bass_exec_p.bind(
            *operands,
            out_avals=tuple(out_avals),
            in_names=all_in_names,
            out_names=tuple(out_names),
            lowering_input_output_aliases=(),
            sim_require_finite=True,
            sim_require_nnan=True,
            nc=nc,
        )
        return tuple(outs)

    devices = jax.devices()[:NCORES]
    assert len(devices) == NCORES, f"need {NCORES} cores, got {len(jax.devices())}"
    mesh = Mesh(np.asarray(devices), ("core",))
    in_specs = (PartitionSpec("core"),) * (n_params + len(out_names))
    out_specs = (PartitionSpec("core"),) * len(out_names)
    fn = jax.jit(
        shard_map(
            _body, mesh=mesh, in_specs=in_specs, out_specs=out_specs, check_rep=False
        ),
        donate_argnums=donate,
        keep_unused=True,
    )
    sharding = NamedSharding(mesh, PartitionSpec("core"))

    # on-device zero buffers for the donated outputs (kernel writes every
    # element, but the custom-call path wants donated operands to reuse)
    def _zeros():
        import jax.numpy as jnp

        return tuple(
            jnp.zeros((NCORES * s[0],) + tuple(s[1:]), dt) for (s, dt) in out_shapes
        )

    zeros_fn = jax.jit(_zeros, out_shardings=(sharding,) * len(out_shapes))

    runner = (jax, fn, zeros_fn, in_names, out_names, sharding)
    _RUNNER_CACHE["runner"] = runner
    return runner


def kernel(**inputs):
    global LAST_EXEC_S
    jax, fn, zeros_fn, in_names, out_names, sharding = _get_runner()

    in_maps = _shard_inputs(inputs)
    concat = [
        jax.device_put(
            np.concatenate([in_maps[c][nm] for c in range(NCORES)], axis=0), sharding
        )
        for nm in in_names
    ]
    zouts = zeros_fn()
    jax.block_until_ready((concat, zouts))

    t0 = time.perf_counter()
    outs = fn(*concat, *zouts)
    outs = jax.block_until_ready(outs)
    LAST_EXEC_S = time.perf_counter() - t0

    res = {nm: np.asarray(o) for nm, o in zip(out_names, outs)}
    # attn: [8*2, L, L]; core d rows [2d, 2d+1] are heads (2*(d%4), 2*(d%4)+1)
    # of batch d//4. attn_view[h*B + b] = global row 8*b + h.
    ga = res["attn"]
    idx = [8 * b + h for h in range(H) for b in range(B)]
    attn_view = ga[idx]
    # outp: per-core [HPC, DV, L] partial outT per head
    go = res["outp"].reshape(NCORES, HPC, DV, L)
    fc_b = np.asarray(inputs["fc_b"], dtype=np.float32)
    out = np.empty((B, L, DV), dtype=np.float32)
    for b in range(B):
        acc = go[4 * b : 4 * b + 4].sum(axis=(0, 1))  # [DV, L]
        out[b] = acc.T + fc_b
    return out, attn_view
